# revision 14
# baseline (speedup 1.0000x reference)
"""Multi-head attention forward on 8 Trainium2 NeuronCores.

Problem (hardcoded): B=4, N=M=2048, D=1024, H=16, HS=64, OUT=1024, fp32.

Sharding: 8 cores = 4 batches x 2 head-groups of 8 heads. Each core
computes a partial output [2048, 1024] = sum over its 8 heads of
softmax((X_q Wq_h)(X_k Wk_h)^T / 8) (X_v Wv_h) Wo_h.  Host sums the two
head-group partials per batch and adds the projection bias.

Host-side prep: x tensors are transposed to [D, N] and converted to
bf16 (so no on-chip transposes are needed); W's are pre-arranged into
the SBUF layouts and converted to bf16.

Per-core kernel:
  1. QKV projections: stationary = W pair-column [d-slice, 128] reused
     across four F=512 moving chunks of xT (amortizes LDWEIGHTS);
     V projection: stationary = xT m-tile, moving = Wv [d-slice, 512].
     PSUM f32 accumulate over 8 d-slices, evict to bf16:
     qt/kt pair-stacked [128, 2048], v_all [128, 8 heads, 65] with a
     ones column at 64 (softmax denominator).
  2. Attention per (pair, 512-chunk): the two heads of a pair run
     CONCURRENTLY as row-tiled matmuls (tile positions (0,0)/(64,0):
     head A uses PE rows 0-63, head B rows 64-127) into adjacent PSUM
     banks; ONE Exp activation [128, 2, 512] (F=1024) per m-tile covers
     both heads; ctx accumulation per head with the [v|1] stationary.
     Pace is set by ScalarE's Exp (~1.15us per m-tile).
  3. Deferred softmax normalization: one reciprocal over all head
     denominators, PE broadcast via head-select masks, DVE multiply.
  4. Output projection: lhsT = ctxn pair n-block (K=128), rhs = Wo pair,
     stationary reused across both 512-wide output chunks.
"""

import os
import sys

import numpy as np

for _p in ("/opt/trn_rl_repo",):
    if _p not in sys.path and os.path.isdir(_p):
        sys.path.insert(0, _p)

B, N, M, D = 4, 2048, 2048, 1024
H, HS, OUT = 16, 64, 1024
HL = 8           # heads per core
P = 128
NPAIR = HL // 2  # head pairs per core
DT = D // P      # 8 d-tiles
NT = N // P      # 16 n-tiles
MT = M // P      # 16 m-tiles
C = 512          # attention n-chunk width
NC = N // C      # 4 chunks


def build_mha(tc, ins, out_ap):
    import contextlib

    from concourse import mybir

    nc = tc.nc
    f32 = mybir.dt.float32
    f32r = mybir.dt.float32r
    bf16 = mybir.dt.bfloat16

    xq, xk, xv = ins["xq"], ins["xk"], ins["xv"]
    wq, wk, wv, wo = ins["wq"], ins["wk"], ins["wv"], ins["wo"]

    with contextlib.ExitStack() as ctx:
        # ---- constant tiles ----
        const = ctx.enter_context(tc.tile_pool(name="const", bufs=1))
        identity = const.tile([P, P], f32)
        from concourse.masks import make_identity
        make_identity(nc, identity)
        identity_bf = const.tile([P, P], bf16)
        nc.vector.tensor_copy(identity_bf[:], identity[:])
        ones_bf = const.tile([P, HL, 1], bf16)
        nc.vector.memset(ones_bf[:], 1.0)
        # head-select masks: hmask[0:2, s, :] is 1 on partition s, else 0.
        # K=2 lhsT for broadcasting one pair-row's denominator to 64
        # partitions.
        hmask_f32 = const.tile([2, 2, 64], f32)
        nc.gpsimd.memset(hmask_f32[:], 0.0)
        nc.gpsimd.affine_select(
            out=hmask_f32[:],
            in_=hmask_f32[:],
            compare_op=mybir.AluOpType.not_equal,
            fill=1.0,
            base=0,
            pattern=[[-1, 2], [0, 64]],
            channel_multiplier=1,
        )


        # ---- persistent activations ----
        act_pool = ctx.enter_context(tc.tile_pool(name="acts", bufs=1))
        qt = [act_pool.tile([P, N], bf16, name=f"qt{p}", tag=f"qt{p}")
              for p in range(NPAIR)]
        kt = [act_pool.tile([P, M], bf16, name=f"kt{p}", tag=f"kt{p}")
              for p in range(NPAIR)]
        v_all = [act_pool.tile([P, HL, 65], bf16, name=f"v{t}", tag=f"v{t}")
                 for t in range(MT)]
        ctxn = [act_pool.tile([P, N], bf16, name=f"ctxn{p}", tag=f"ctxn{p}")
                for p in range(NPAIR)]
        sums_pr = [act_pool.tile([2, N], f32, name=f"sums{p}", tag=f"sums{p}")
                   for p in range(NPAIR)]
        wo_sb = act_pool.tile([P, NPAIR, OUT], bf16, name="wo_sb", tag="wo_sb")
        nc.sync.dma_start(wo_sb[:], wo[:, :, :])

        # ---- phase 1: load + QKV projections ----
        # All projection matmuls are K=64 row-split pairs at tile
        # positions (0,0)/(64,0): the two halves of each d-slice run
        # CONCURRENTLY in the PE array and each half's LDWEIGHTS hides
        # under the other half's matmul. The halves accumulate in
        # separate PSUM tiles, summed on DVE during eviction.
        with tc.tile_pool(name="x_sb", bufs=2) as x_pool, \
             tc.tile_pool(name="w_sb", bufs=2) as w_pool:

            def load_xw(x_dram, w_dram):
                # per-d-slice x tiles so matmuls start as soon as each
                # slice's DMA lands
                x_sb = [x_pool.tile([P, N], bf16, name=f"x{d}", tag=f"x{d}")
                        for d in range(DT)]
                w_sb = w_pool.tile([P, DT, HL * HS], bf16, name="w_sb", tag="w_sb")
                for dt_i in range(DT):
                    dsl = slice(dt_i * P, (dt_i + 1) * P)
                    nc.sync.dma_start(x_sb[dt_i][:], x_dram[dsl, :])
                    nc.sync.dma_start(w_sb[:, dt_i, :], w_dram[dsl, :])
                return x_sb, w_sb

            # V first (attention needs all of v_all). d-slice-outer within
            # groups of 4 m-tiles: compute starts after the first x DMA.
            xv_sb, wv_sb = load_xw(xv, wv)
            with tc.tile_pool(name="pv_psum", bufs=4, space="PSUM") as pv_psum:
                for g in range(MT // 4):
                    pse = [pv_psum.tile([P, HL * HS], f32, name="vE", tag="vE")
                           for _ in range(4)]
                    pso = [pv_psum.tile([P, HL * HS], f32, name="vO", tag="vO")
                           for _ in range(4)]
                    for dt_i in range(DT):
                        for mi in range(4):
                            t = 4 * g + mi
                            msl = slice(t * P, (t + 1) * P)
                            nc.tensor.matmul(
                                pse[mi][:], xv_sb[dt_i][0:64, msl],
                                wv_sb[0:64, dt_i, :],
                                start=(dt_i == 0), stop=(dt_i == DT - 1))
                            nc.tensor.matmul(
                                pso[mi][:], xv_sb[dt_i][64:128, msl],
                                wv_sb[64:128, dt_i, :],
                                start=(dt_i == 0), stop=(dt_i == DT - 1))
                    for mi in range(4):
                        t = 4 * g + mi
                        vstage = x_pool.tile([P, HL * HS], f32,
                                             name="vstage", tag="vstage")
                        nc.vector.tensor_copy(vstage[:], pso[mi][:])
                        nc.vector.tensor_add(
                            v_all[t][:, :, 0:64],
                            pse[mi][:].rearrange("p (h o) -> p h o", h=HL),
                            vstage[:].rearrange("p (h o) -> p h o", h=HL))
                        nc.vector.tensor_copy(v_all[t][:, :, 64:65], ones_bf[:])

            def qk_proj(x_sb, w_sb, dst, pj_psum):
                for p in range(NPAIR):
                    for half in range(2):
                        hsl = slice(half * 1024, (half + 1) * 1024)
                        pse = pj_psum.tile([P, 1024], f32, name="qkE", tag="qkE")
                        pso = pj_psum.tile([P, 1024], f32, name="qkO", tag="qkO")
                        for dt_i in range(DT):
                            for cc in range(2):
                                xs = x_sb[dt_i][:, half * 1024 + cc * C:
                                                half * 1024 + (cc + 1) * C]
                                nc.tensor.matmul(
                                    pse[:, cc * C:(cc + 1) * C],
                                    w_sb[0:64, dt_i, p * P:(p + 1) * P],
                                    xs[0:64, :],
                                    start=(dt_i == 0), stop=(dt_i == DT - 1),
                                )
                                nc.tensor.matmul(
                                    pso[:, cc * C:(cc + 1) * C],
                                    w_sb[64:128, dt_i, p * P:(p + 1) * P],
                                    xs[64:128, :],
                                    start=(dt_i == 0), stop=(dt_i == DT - 1),
                                )
                        qstage = x_pool.tile([P, 1024], f32,
                                             name="qstage", tag="qstage")
                        nc.vector.tensor_copy(qstage[:], pso[:])
                        nc.vector.tensor_add(dst[p][:, hsl], pse[:], qstage[:])

            with tc.tile_pool(name="pj_psum", bufs=2, space="PSUM") as pj_psum:
                xq_sb, wq_sb = load_xw(xq, wq)
                qk_proj(xq_sb, wq_sb, qt, pj_psum)
                xk_sb, wk_sb = load_xw(xk, wk)
                qk_proj(xk_sb, wk_sb, kt, pj_psum)

        # ---- phase 2: attention (software-pipelined across chunks) ----
        PIPE = 2
        with tc.tile_pool(name="et", bufs=5) as et_pool, \
             tc.tile_pool(name="tmp", bufs=3) as tmp_pool, \
             tc.tile_pool(name="lg_psum", bufs=2, space="PSUM") as lg_psum, \
             tc.tile_pool(name="shbc_psum", bufs=1, space="PSUM") as shbc_psum, \
             tc.tile_pool(name="ctx_psum", bufs=3, space="PSUM") as ctx_psum:

            def make_evict(p, c, cps):
                # Evict closures: ctx rows UN-normalized into pair-stacked
                # ctxn (odd head via bf16 PE shift to partitions 64:128);
                # denominator rows staged to sums via partition-hop DMA.
                # Emitted DURING the next chunk's t-loop so the PE / ACT
                # pipeline never drains at a chunk boundary. Split in two:
                # the DVE part frees the cps banks ASAP, the PE shift part
                # runs later.
                csl = slice(c * C, (c + 1) * C)
                tmp = tmp_pool.tile([64, C], bf16, name="ctmp", tag="ctmp")

                def evict_dve():
                    stage = tmp_pool.tile([P, 2, C], f32, name="sstage", tag="sstage")
                    nc.vector.tensor_copy(stage[64:65, 0, :], cps[0][64:65, :])
                    nc.vector.tensor_copy(stage[64:65, 1, :], cps[1][64:65, :])
                    nc.sync.dma_start(sums_pr[p][0:1, csl], stage[64:65, 0, :])
                    nc.sync.dma_start(sums_pr[p][1:2, csl], stage[64:65, 1, :])
                    nc.vector.tensor_copy(ctxn[p][0:64, csl], cps[0][0:64, :])
                    nc.vector.tensor_copy(tmp[:], cps[1][0:64, :])

                def evict_shift():
                    sh = shbc_psum.tile([P, C], f32, name="shbc", tag="shbc")
                    nc.tensor.matmul(
                        sh[64:128, :], identity_bf[0:64, 0:64], tmp[:],
                        start=True, stop=True)
                    nc.vector.tensor_copy(ctxn[p][64:128, csl], sh[64:128, :])
                return [evict_dve, evict_shift]

            def make_norm(p):
                # Per-pair deferred softmax normalization, split into
                # 512-wide quarters: reciprocal of the pair's denominator
                # rows, PE broadcast to 64 partitions via pair-row select
                # masks, multiply into ctxn. Emitted during the NEXT
                # pair's attention (PE slack absorbs it).
                def quarter(cc, recip):
                    qsl = slice(cc * C, (cc + 1) * C)

                    def norm_q():
                        if recip:
                            nc.vector.reciprocal(sums_pr[p][:], sums_pr[p][:])
                        bc = shbc_psum.tile([P, C], f32, name="shbc", tag="shbc")
                        for s in range(2):
                            nc.tensor.matmul(
                                bc[s * 64:(s + 1) * 64, :],
                                hmask_f32[:, s, :],
                                sums_pr[p][:, qsl],
                                start=True, stop=True,
                            )
                        nc.vector.tensor_mul(
                            ctxn[p][:, qsl], ctxn[p][:, qsl], bc[:])
                    return norm_q
                return [quarter(cc, cc == 0) for cc in range(NC)]

            pending = []  # deferred work, emitted inside later t-loops
            SLOTS = (1, 3, 6, 9, 12)

            for p in range(NPAIR):
                hA, hB = 2 * p, 2 * p + 1
                for c in range(NC):
                    csl = slice(c * C, (c + 1) * C)
                    cps = {
                        0: ctx_psum.tile([65, C], f32, name="cpsA", tag="cps"),
                        1: ctx_psum.tile([65, C], f32, name="cpsB", tag="cps"),
                    }
                    ets = {}

                    def emit_logits(t):
                        tsl = slice(t * P, (t + 1) * P)
                        lg = lg_psum.tile([P, 2, C], f32, name="lg", tag="lg")
                        nc.tensor.matmul(
                            lg[:, 0, :], kt[p][0:64, tsl], qt[p][0:64, csl],
                            start=True, stop=True)
                        nc.tensor.matmul(
                            lg[:, 1, :], kt[p][64:128, tsl], qt[p][64:128, csl],
                            start=True, stop=True)
                        et = et_pool.tile([P, 2, C], bf16, name="et", tag="et")
                        nc.scalar.activation(
                            et[:], lg[:], mybir.ActivationFunctionType.Exp,
                            scale=0.125)
                        ets[t] = et

                    def emit_ctx(t):
                        et = ets.pop(t)
                        nc.tensor.matmul(
                            cps[0][:], v_all[t][:, hA, :], et[:, 0, :],
                            start=(t == 0), stop=(t == MT - 1))
                        nc.tensor.matmul(
                            cps[1][:], v_all[t][:, hB, :], et[:, 1, :],
                            start=(t == 0), stop=(t == MT - 1))

                    for t in range(MT):
                        emit_logits(t)
                        if t in SLOTS and pending:
                            pending.pop(0)()
                        if t >= PIPE:
                            emit_ctx(t - PIPE)
                    for t in range(MT - PIPE, MT):
                        emit_ctx(t)

                    pending.extend(make_evict(p, c, cps))
                if p > 0:
                    pending.extend(make_norm(p - 1))
            for w in pending:
                w()
            for w in make_norm(NPAIR - 1):
                w()

        # ---- phase 3: output projection (transposed: outT = Wo^T ctx) ----
        # stationary = wo column block [128=(s,hs), 128], reused across the
        # four n-chunks (4 matmuls per LDWEIGHTS); accumulate over pairs.
        # Host transposes [OUT, N] -> [N, OUT].
        with tc.tile_pool(name="out_psum", bufs=8, space="PSUM") as out_psum, \
             tc.tile_pool(name="out_sb", bufs=2) as out_pool:
            for oc in range(OUT // P):
                ops = [out_psum.tile([P, C], f32, name=f"ops{nk}", tag="ops")
                       for nk in range(NC)]
                for p in range(NPAIR):
                    for nk in range(NC):
                        nc.tensor.matmul(
                            ops[nk][:],
                            wo_sb[:, p, oc * P:(oc + 1) * P],
                            ctxn[p][:, nk * C:(nk + 1) * C],
                            start=(p == 0), stop=(p == NPAIR - 1),
                        )
                ot = out_pool.tile([P, N], f32, name="ot", tag="ot")
                for nk in range(NC):
                    nc.vector.tensor_copy(ot[:, nk * C:(nk + 1) * C], ops[nk][:])
                nc.sync.dma_start(out_ap[oc * P:(oc + 1) * P, :], ot[:])


def build_nc():
    import concourse.bacc as bacc
    import concourse.tile as tile
    from concourse import mybir

    nc = bacc.Bacc("TRN2", target_bir_lowering=False, debug=False)
    f32 = mybir.dt.float32
    bf16 = mybir.dt.bfloat16
    ins = {
        "xq": nc.dram_tensor("xq", (D, N), bf16, kind="ExternalInput").ap(),
        "xk": nc.dram_tensor("xk", (D, M), bf16, kind="ExternalInput").ap(),
        "xv": nc.dram_tensor("xv", (D, M), bf16, kind="ExternalInput").ap(),
        "wq": nc.dram_tensor("wq", (D, HL * HS), bf16, kind="ExternalInput").ap(),
        "wk": nc.dram_tensor("wk", (D, HL * HS), bf16, kind="ExternalInput").ap(),
        "wv": nc.dram_tensor("wv", (D, HL * HS), bf16, kind="ExternalInput").ap(),
        "wo": nc.dram_tensor("wo", (P, NPAIR, OUT), bf16, kind="ExternalInput").ap(),
    }
    out_ap = nc.dram_tensor("out", (OUT, N), f32, kind="ExternalOutput").ap()
    with tile.TileContext(nc) as tc:
        build_mha(tc, ins, out_ap)
    nc.compile()
    return nc


def make_in_maps(inputs):
    import ml_dtypes
    bf16 = ml_dtypes.bfloat16

    q = np.asarray(inputs["query"], dtype=np.float32)
    k = np.asarray(inputs["key"], dtype=np.float32)
    v = np.asarray(inputs["value"], dtype=np.float32)
    wq = np.asarray(inputs["query_kernel"], dtype=np.float32)
    wk = np.asarray(inputs["key_kernel"], dtype=np.float32)
    wv = np.asarray(inputs["value_kernel"], dtype=np.float32)
    wo = np.asarray(inputs["projection_kernel"], dtype=np.float32)

    # [H, D, HS] -> per head-group [D, HL*HS] bf16
    def wlay(w, hs):
        return np.ascontiguousarray(
            w[hs].transpose(1, 0, 2).reshape(D, HL * HS)).astype(bf16)

    # [H, HS, OUT] -> per head-group [128=(s,o), NPAIR, OUT] bf16
    def wolay(w, hs):
        return np.ascontiguousarray(
            w[hs].reshape(NPAIR, 2, HS, OUT).transpose(1, 2, 0, 3)
            .reshape(P, NPAIR, OUT)).astype(bf16)

    in_maps = []
    for cc in range(8):
        b, hg = divmod(cc, 2)
        hs = slice(hg * HL, (hg + 1) * HL)
        in_maps.append({
            "xq": np.ascontiguousarray(q[b].T).astype(bf16),
            "xk": np.ascontiguousarray(k[b].T).astype(bf16),
            "xv": np.ascontiguousarray(v[b].T).astype(bf16),
            "wq": wlay(wq, hs),
            "wk": wlay(wk, hs),
            "wv": wlay(wv, hs),
            "wo": wolay(wo, hs),
        })
    return in_maps


def combine(results, bias):
    # per-core output is transposed [OUT, N]
    out = np.empty((B, N, OUT), dtype=np.float32)
    for b in range(B):
        out[b] = (results[2 * b]["out"] + results[2 * b + 1]["out"]).T
    out += np.asarray(bias, dtype=np.float32)[None, None, :]
    return out


_NC_CACHE = None
_LDW_PATCHED = False


def _enable_ldw_opt():
    """No-op: walrus --enable-ldw-opt=true rejects tile_position'd
    LDWEIGHTS ("InstLdweights is not compatible with LDW optimization"),
    and this kernel's row-tiled attention matmuls need tile positions.
    Kept for test.py compatibility."""
    return


def kernel(**inputs):
    global _NC_CACHE
    from concourse import bass_utils
    _enable_ldw_opt()

    if _NC_CACHE is None:
        _NC_CACHE = build_nc()
    nc = _NC_CACHE
    in_maps = make_in_maps(inputs)
    res = bass_utils.run_bass_kernel_spmd(nc, in_maps, core_ids=list(range(8)))
    return combine(res.results, inputs["projection_bias"])


# revision 15
# speedup vs baseline: 1.0189x; 1.0189x over previous
"""Multi-head attention forward on 8 Trainium2 NeuronCores.

Problem (hardcoded): B=4, N=M=2048, D=1024, H=16, HS=64, OUT=1024, fp32.

Sharding: 8 cores = 4 batches x 2 head-groups of 8 heads. Each core
computes a partial output [2048, 1024] = sum over its 8 heads of
softmax((X_q Wq_h)(X_k Wk_h)^T / 8) (X_v Wv_h) Wo_h.  Host sums the two
head-group partials per batch and adds the projection bias.

Host-side prep: x tensors are transposed to [D, N] and converted to
bf16 (so no on-chip transposes are needed); W's are pre-arranged into
the SBUF layouts and converted to bf16.

Per-core kernel:
  1. QKV projections: stationary = W pair-column [d-slice, 128] reused
     across four F=512 moving chunks of xT (amortizes LDWEIGHTS);
     V projection: stationary = xT m-tile, moving = Wv [d-slice, 512].
     PSUM f32 accumulate over 8 d-slices, evict to bf16:
     qt/kt pair-stacked [128, 2048], v_all [128, 8 heads, 65] with a
     ones column at 64 (softmax denominator).
  2. Attention per (pair, 512-chunk): the two heads of a pair run
     CONCURRENTLY as row-tiled matmuls (tile positions (0,0)/(64,0):
     head A uses PE rows 0-63, head B rows 64-127) into adjacent PSUM
     banks; ONE Exp activation [128, 2, 512] (F=1024) per m-tile covers
     both heads; ctx accumulation per head with the [v|1] stationary.
     Pace is set by ScalarE's Exp (~1.15us per m-tile).
  3. Deferred softmax normalization: one reciprocal over all head
     denominators, PE broadcast via head-select masks, DVE multiply.
  4. Output projection: lhsT = ctxn pair n-block (K=128), rhs = Wo pair,
     stationary reused across both 512-wide output chunks.
"""

import os
import sys

import numpy as np

for _p in ("/opt/trn_rl_repo",):
    if _p not in sys.path and os.path.isdir(_p):
        sys.path.insert(0, _p)

B, N, M, D = 4, 2048, 2048, 1024
H, HS, OUT = 16, 64, 1024
HL = 8           # heads per core
P = 128
NPAIR = HL // 2  # head pairs per core
DT = D // P      # 8 d-tiles
NT = N // P      # 16 n-tiles
MT = M // P      # 16 m-tiles
C = 512          # attention n-chunk width
NC = N // C      # 4 chunks


def build_mha(tc, ins, out_ap):
    import contextlib

    from concourse import mybir

    nc = tc.nc
    f32 = mybir.dt.float32
    f32r = mybir.dt.float32r
    bf16 = mybir.dt.bfloat16

    xq, xk, xv = ins["xq"], ins["xk"], ins["xv"]
    wq, wk, wv, wo = ins["wq"], ins["wk"], ins["wv"], ins["wo"]

    with contextlib.ExitStack() as ctx:
        # ---- constant tiles ----
        const = ctx.enter_context(tc.tile_pool(name="const", bufs=1))
        identity = const.tile([P, P], f32)
        from concourse.masks import make_identity
        make_identity(nc, identity)
        identity_bf = const.tile([P, P], bf16)
        nc.vector.tensor_copy(identity_bf[:], identity[:])
        ones_bf = const.tile([P, HL, 1], bf16)
        nc.vector.memset(ones_bf[:], 1.0)
        # head-select masks: hmask[0:2, s, :] is 1 on partition s, else 0.
        # K=2 lhsT for broadcasting one pair-row's denominator to 64
        # partitions.
        hmask_f32 = const.tile([2, 2, 64], f32)
        nc.gpsimd.memset(hmask_f32[:], 0.0)
        nc.gpsimd.affine_select(
            out=hmask_f32[:],
            in_=hmask_f32[:],
            compare_op=mybir.AluOpType.not_equal,
            fill=1.0,
            base=0,
            pattern=[[-1, 2], [0, 64]],
            channel_multiplier=1,
        )


        # ---- persistent activations ----
        act_pool = ctx.enter_context(tc.tile_pool(name="acts", bufs=1))
        qt = [act_pool.tile([P, N], bf16, name=f"qt{p}", tag=f"qt{p}")
              for p in range(NPAIR)]
        kt = [act_pool.tile([P, M], bf16, name=f"kt{p}", tag=f"kt{p}")
              for p in range(NPAIR)]
        v_all = [act_pool.tile([P, HL, 65], bf16, name=f"v{t}", tag=f"v{t}")
                 for t in range(MT)]
        ctxn = [act_pool.tile([P, N], bf16, name=f"ctxn{p}", tag=f"ctxn{p}")
                for p in range(NPAIR)]
        sums_pr = [act_pool.tile([2, N], f32, name=f"sums{p}", tag=f"sums{p}")
                   for p in range(NPAIR)]
        wo_sb = act_pool.tile([P, NPAIR, OUT], bf16, name="wo_sb", tag="wo_sb")
        nc.sync.dma_start(wo_sb[:], wo[:, :, :])

        # ---- phase 1: load + QKV projections ----
        # All projection matmuls are K=64 row-split pairs at tile
        # positions (0,0)/(64,0): the two halves of each d-slice run
        # CONCURRENTLY in the PE array and each half's LDWEIGHTS hides
        # under the other half's matmul. The halves accumulate in
        # separate PSUM tiles, summed on DVE during eviction.
        with tc.tile_pool(name="x_sb", bufs=2) as x_pool, \
             tc.tile_pool(name="w_sb", bufs=2) as w_pool:

            def load_xw(x_dram, w_dram):
                # per-d-slice x tiles so matmuls start as soon as each
                # slice's DMA lands
                x_sb = [x_pool.tile([P, N], bf16, name=f"x{d}", tag=f"x{d}")
                        for d in range(DT)]
                w_sb = w_pool.tile([P, DT, HL * HS], bf16, name="w_sb", tag="w_sb")
                for dt_i in range(DT):
                    dsl = slice(dt_i * P, (dt_i + 1) * P)
                    nc.sync.dma_start(x_sb[dt_i][:], x_dram[dsl, :])
                    nc.sync.dma_start(w_sb[:, dt_i, :], w_dram[dsl, :])
                return x_sb, w_sb

            # V first (attention needs all of v_all). d-slice-outer within
            # groups of 4 m-tiles: compute starts after the first x DMA.
            xv_sb, wv_sb = load_xw(xv, wv)
            with tc.tile_pool(name="pv_psum", bufs=4, space="PSUM") as pv_psum:
                for g in range(MT // 4):
                    pse = [pv_psum.tile([P, HL * HS], f32, name="vE", tag="vE")
                           for _ in range(4)]
                    pso = [pv_psum.tile([P, HL * HS], f32, name="vO", tag="vO")
                           for _ in range(4)]
                    for dt_i in range(DT):
                        for mi in range(4):
                            t = 4 * g + mi
                            msl = slice(t * P, (t + 1) * P)
                            nc.tensor.matmul(
                                pse[mi][:], xv_sb[dt_i][0:64, msl],
                                wv_sb[0:64, dt_i, :],
                                start=(dt_i == 0), stop=(dt_i == DT - 1))
                            nc.tensor.matmul(
                                pso[mi][:], xv_sb[dt_i][64:128, msl],
                                wv_sb[64:128, dt_i, :],
                                start=(dt_i == 0), stop=(dt_i == DT - 1))
                    for mi in range(4):
                        t = 4 * g + mi
                        vstage = x_pool.tile([P, HL * HS], f32,
                                             name="vstage", tag="vstage")
                        nc.vector.tensor_copy(vstage[:], pso[mi][:])
                        nc.vector.tensor_add(
                            v_all[t][:, :, 0:64],
                            pse[mi][:].rearrange("p (h o) -> p h o", h=HL),
                            vstage[:].rearrange("p (h o) -> p h o", h=HL))
                        nc.vector.tensor_copy(v_all[t][:, :, 64:65], ones_bf[:])

            def qk_proj(x_sb, w_sb, dst, pj_psum):
                for p in range(NPAIR):
                    for half in range(2):
                        hsl = slice(half * 1024, (half + 1) * 1024)
                        ps = pj_psum.tile([P, 1024], f32, name="qk_ps", tag="qk_ps")
                        for dt_i in range(DT):
                            for cc in range(2):
                                nc.tensor.matmul(
                                    ps[:, cc * C:(cc + 1) * C],
                                    w_sb[:, dt_i, p * P:(p + 1) * P],
                                    x_sb[dt_i][:, half * 1024 + cc * C:
                                               half * 1024 + (cc + 1) * C],
                                    start=(dt_i == 0), stop=(dt_i == DT - 1),
                                )
                        nc.vector.tensor_copy(dst[p][:, hsl], ps[:])

            with tc.tile_pool(name="pj_psum", bufs=2, space="PSUM") as pj_psum:
                xq_sb, wq_sb = load_xw(xq, wq)
                qk_proj(xq_sb, wq_sb, qt, pj_psum)
                xk_sb, wk_sb = load_xw(xk, wk)
                qk_proj(xk_sb, wk_sb, kt, pj_psum)

        # ---- phase 2: attention (software-pipelined across chunks) ----
        PIPE = 2
        with tc.tile_pool(name="et", bufs=5) as et_pool, \
             tc.tile_pool(name="tmp", bufs=3) as tmp_pool, \
             tc.tile_pool(name="lg_psum", bufs=2, space="PSUM") as lg_psum, \
             tc.tile_pool(name="shbc_psum", bufs=1, space="PSUM") as shbc_psum, \
             tc.tile_pool(name="ctx_psum", bufs=3, space="PSUM") as ctx_psum:

            def make_evict(p, c, cps):
                # Evict closures: ctx rows UN-normalized into pair-stacked
                # ctxn (odd head via bf16 PE shift to partitions 64:128);
                # denominator rows staged to sums via partition-hop DMA.
                # Emitted DURING the next chunk's t-loop so the PE / ACT
                # pipeline never drains at a chunk boundary. Split in two:
                # the DVE part frees the cps banks ASAP, the PE shift part
                # runs later.
                csl = slice(c * C, (c + 1) * C)
                tmp = tmp_pool.tile([64, C], bf16, name="ctmp", tag="ctmp")

                def evict_dve():
                    stage = tmp_pool.tile([P, 2, C], f32, name="sstage", tag="sstage")
                    nc.vector.tensor_copy(stage[64:65, 0, :], cps[0][64:65, :])
                    nc.vector.tensor_copy(stage[64:65, 1, :], cps[1][64:65, :])
                    nc.sync.dma_start(sums_pr[p][0:1, csl], stage[64:65, 0, :])
                    nc.sync.dma_start(sums_pr[p][1:2, csl], stage[64:65, 1, :])
                    nc.vector.tensor_copy(ctxn[p][0:64, csl], cps[0][0:64, :])
                    nc.vector.tensor_copy(tmp[:], cps[1][0:64, :])

                def evict_shift():
                    sh = shbc_psum.tile([P, C], f32, name="shbc", tag="shbc")
                    nc.tensor.matmul(
                        sh[64:128, :], identity_bf[0:64, 0:64], tmp[:],
                        start=True, stop=True)
                    nc.vector.tensor_copy(ctxn[p][64:128, csl], sh[64:128, :])
                return [evict_dve, evict_shift]

            def make_norm(p):
                # Per-pair deferred softmax normalization, split into
                # 512-wide quarters: reciprocal of the pair's denominator
                # rows, PE broadcast to 64 partitions via pair-row select
                # masks, multiply into ctxn. Emitted during the NEXT
                # pair's attention (PE slack absorbs it).
                def quarter(cc, recip):
                    qsl = slice(cc * C, (cc + 1) * C)

                    def norm_q():
                        if recip:
                            nc.vector.reciprocal(sums_pr[p][:], sums_pr[p][:])
                        bc = shbc_psum.tile([P, C], f32, name="shbc", tag="shbc")
                        for s in range(2):
                            nc.tensor.matmul(
                                bc[s * 64:(s + 1) * 64, :],
                                hmask_f32[:, s, :],
                                sums_pr[p][:, qsl],
                                start=True, stop=True,
                            )
                        nc.vector.tensor_mul(
                            ctxn[p][:, qsl], ctxn[p][:, qsl], bc[:])
                    return norm_q
                return [quarter(cc, cc == 0) for cc in range(NC)]

            pending = []  # deferred work, emitted inside later t-loops
            SLOTS = (1, 3, 6, 9, 12)

            for p in range(NPAIR):
                hA, hB = 2 * p, 2 * p + 1
                for c in range(NC):
                    csl = slice(c * C, (c + 1) * C)
                    cps = {
                        0: ctx_psum.tile([65, C], f32, name="cpsA", tag="cps"),
                        1: ctx_psum.tile([65, C], f32, name="cpsB", tag="cps"),
                    }
                    ets = {}

                    def emit_logits(t):
                        tsl = slice(t * P, (t + 1) * P)
                        lg = lg_psum.tile([P, 2, C], f32, name="lg", tag="lg")
                        nc.tensor.matmul(
                            lg[:, 0, :], kt[p][0:64, tsl], qt[p][0:64, csl],
                            start=True, stop=True)
                        nc.tensor.matmul(
                            lg[:, 1, :], kt[p][64:128, tsl], qt[p][64:128, csl],
                            start=True, stop=True)
                        et = et_pool.tile([P, 2, C], bf16, name="et", tag="et")
                        nc.scalar.activation(
                            et[:], lg[:], mybir.ActivationFunctionType.Exp,
                            scale=0.125)
                        ets[t] = et

                    def emit_ctx(t):
                        et = ets.pop(t)
                        nc.tensor.matmul(
                            cps[0][:], v_all[t][:, hA, :], et[:, 0, :],
                            start=(t == 0), stop=(t == MT - 1))
                        nc.tensor.matmul(
                            cps[1][:], v_all[t][:, hB, :], et[:, 1, :],
                            start=(t == 0), stop=(t == MT - 1))

                    for t in range(MT):
                        emit_logits(t)
                        if t in SLOTS and pending:
                            pending.pop(0)()
                        if t >= PIPE:
                            emit_ctx(t - PIPE)
                    for t in range(MT - PIPE, MT):
                        emit_ctx(t)

                    pending.extend(make_evict(p, c, cps))
                if p > 0:
                    pending.extend(make_norm(p - 1))
            for w in pending:
                w()
            for w in make_norm(NPAIR - 1):
                w()

        # ---- phase 3: output projection (transposed: outT = Wo^T ctx) ----
        # stationary = wo column block [128=(s,hs), 128], reused across the
        # four n-chunks (4 matmuls per LDWEIGHTS); accumulate over pairs.
        # Host transposes [OUT, N] -> [N, OUT].
        with tc.tile_pool(name="out_psum", bufs=8, space="PSUM") as out_psum, \
             tc.tile_pool(name="out_sb", bufs=2) as out_pool:
            for oc in range(OUT // P):
                ops = [out_psum.tile([P, C], f32, name=f"ops{nk}", tag="ops")
                       for nk in range(NC)]
                for p in range(NPAIR):
                    for nk in range(NC):
                        nc.tensor.matmul(
                            ops[nk][:],
                            wo_sb[:, p, oc * P:(oc + 1) * P],
                            ctxn[p][:, nk * C:(nk + 1) * C],
                            start=(p == 0), stop=(p == NPAIR - 1),
                        )
                ot = out_pool.tile([P, N], f32, name="ot", tag="ot")
                for nk in range(NC):
                    nc.vector.tensor_copy(ot[:, nk * C:(nk + 1) * C], ops[nk][:])
                nc.sync.dma_start(out_ap[oc * P:(oc + 1) * P, :], ot[:])


def build_nc():
    import concourse.bacc as bacc
    import concourse.tile as tile
    from concourse import mybir

    nc = bacc.Bacc("TRN2", target_bir_lowering=False, debug=False)
    f32 = mybir.dt.float32
    bf16 = mybir.dt.bfloat16
    ins = {
        "xq": nc.dram_tensor("xq", (D, N), bf16, kind="ExternalInput").ap(),
        "xk": nc.dram_tensor("xk", (D, M), bf16, kind="ExternalInput").ap(),
        "xv": nc.dram_tensor("xv", (D, M), bf16, kind="ExternalInput").ap(),
        "wq": nc.dram_tensor("wq", (D, HL * HS), bf16, kind="ExternalInput").ap(),
        "wk": nc.dram_tensor("wk", (D, HL * HS), bf16, kind="ExternalInput").ap(),
        "wv": nc.dram_tensor("wv", (D, HL * HS), bf16, kind="ExternalInput").ap(),
        "wo": nc.dram_tensor("wo", (P, NPAIR, OUT), bf16, kind="ExternalInput").ap(),
    }
    out_ap = nc.dram_tensor("out", (OUT, N), f32, kind="ExternalOutput").ap()
    with tile.TileContext(nc) as tc:
        build_mha(tc, ins, out_ap)
    nc.compile()
    return nc


def make_in_maps(inputs):
    import ml_dtypes
    bf16 = ml_dtypes.bfloat16

    q = np.asarray(inputs["query"], dtype=np.float32)
    k = np.asarray(inputs["key"], dtype=np.float32)
    v = np.asarray(inputs["value"], dtype=np.float32)
    wq = np.asarray(inputs["query_kernel"], dtype=np.float32)
    wk = np.asarray(inputs["key_kernel"], dtype=np.float32)
    wv = np.asarray(inputs["value_kernel"], dtype=np.float32)
    wo = np.asarray(inputs["projection_kernel"], dtype=np.float32)

    # [H, D, HS] -> per head-group [D, HL*HS] bf16
    def wlay(w, hs):
        return np.ascontiguousarray(
            w[hs].transpose(1, 0, 2).reshape(D, HL * HS)).astype(bf16)

    # [H, HS, OUT] -> per head-group [128=(s,o), NPAIR, OUT] bf16
    def wolay(w, hs):
        return np.ascontiguousarray(
            w[hs].reshape(NPAIR, 2, HS, OUT).transpose(1, 2, 0, 3)
            .reshape(P, NPAIR, OUT)).astype(bf16)

    in_maps = []
    for cc in range(8):
        b, hg = divmod(cc, 2)
        hs = slice(hg * HL, (hg + 1) * HL)
        in_maps.append({
            "xq": np.ascontiguousarray(q[b].T).astype(bf16),
            "xk": np.ascontiguousarray(k[b].T).astype(bf16),
            "xv": np.ascontiguousarray(v[b].T).astype(bf16),
            "wq": wlay(wq, hs),
            "wk": wlay(wk, hs),
            "wv": wlay(wv, hs),
            "wo": wolay(wo, hs),
        })
    return in_maps


def combine(results, bias):
    # per-core output is transposed [OUT, N]
    out = np.empty((B, N, OUT), dtype=np.float32)
    for b in range(B):
        out[b] = (results[2 * b]["out"] + results[2 * b + 1]["out"]).T
    out += np.asarray(bias, dtype=np.float32)[None, None, :]
    return out


_NC_CACHE = None
_LDW_PATCHED = False


def _enable_ldw_opt():
    """No-op: walrus --enable-ldw-opt=true rejects tile_position'd
    LDWEIGHTS ("InstLdweights is not compatible with LDW optimization"),
    and this kernel's row-tiled attention matmuls need tile positions.
    Kept for test.py compatibility."""
    return


def kernel(**inputs):
    global _NC_CACHE
    from concourse import bass_utils
    _enable_ldw_opt()

    if _NC_CACHE is None:
        _NC_CACHE = build_nc()
    nc = _NC_CACHE
    in_maps = make_in_maps(inputs)
    res = bass_utils.run_bass_kernel_spmd(nc, in_maps, core_ids=list(range(8)))
    return combine(res.results, inputs["projection_bias"])


# revision 19
# speedup vs baseline: 1.0932x; 1.0729x over previous
"""Multi-head attention forward on 8 Trainium2 NeuronCores.

Problem (hardcoded): B=4, N=M=2048, D=1024, H=16, HS=64, OUT=1024, fp32.

Sharding: 8 cores = 4 batches x 2 head-groups of 8 heads. Each core
computes a partial output [2048, 1024] = sum over its 8 heads of
softmax((X_q Wq_h)(X_k Wk_h)^T / 8) (X_v Wv_h) Wo_h.  Host sums the two
head-group partials per batch and adds the projection bias.

Host-side prep: x tensors are transposed to [D, N] and converted to
bf16 (so no on-chip transposes are needed); W's are pre-arranged into
the SBUF layouts and converted to bf16.

Per-core kernel:
  1. QKV projections: stationary = W pair-column [d-slice, 128] reused
     across four F=512 moving chunks of xT (amortizes LDWEIGHTS);
     V projection: stationary = xT m-tile, moving = Wv [d-slice, 512].
     PSUM f32 accumulate over 8 d-slices, evict to bf16:
     qt/kt pair-stacked [128, 2048], v_all [128, 8 heads, 65] with a
     ones column at 64 (softmax denominator).
  2. Attention per (pair, 512-chunk): the two heads of a pair run
     CONCURRENTLY as row-tiled matmuls (tile positions (0,0)/(64,0):
     head A uses PE rows 0-63, head B rows 64-127) into adjacent PSUM
     banks; ONE Exp activation [128, 2, 512] (F=1024) per m-tile covers
     both heads; ctx accumulation per head with the [v|1] stationary.
     Pace is set by ScalarE's Exp (~1.15us per m-tile).
  3. Deferred softmax normalization: one reciprocal over all head
     denominators, PE broadcast via head-select masks, DVE multiply.
  4. Output projection: lhsT = ctxn pair n-block (K=128), rhs = Wo pair,
     stationary reused across both 512-wide output chunks.
"""

import os
import sys

import numpy as np

for _p in ("/opt/trn_rl_repo",):
    if _p not in sys.path and os.path.isdir(_p):
        sys.path.insert(0, _p)

B, N, M, D = 4, 2048, 2048, 1024
H, HS, OUT = 16, 64, 1024
HL = 8           # heads per core
P = 128
NPAIR = HL // 2  # head pairs per core
DT = D // P      # 8 d-tiles
NT = N // P      # 16 n-tiles
MT = M // P      # 16 m-tiles
C = 512          # attention n-chunk width
NC = N // C      # 4 chunks


def build_mha(tc, ins, out_ap):
    import contextlib

    from concourse import mybir

    nc = tc.nc
    f32 = mybir.dt.float32
    f32r = mybir.dt.float32r
    bf16 = mybir.dt.bfloat16

    xq, xk, xv = ins["xq"], ins["xk"], ins["xv"]
    wq, wk, wv, wo = ins["wq"], ins["wk"], ins["wv"], ins["wo"]

    with contextlib.ExitStack() as ctx:
        # ---- constant tiles ----
        const = ctx.enter_context(tc.tile_pool(name="const", bufs=1))
        identity = const.tile([P, P], f32)
        from concourse.masks import make_identity
        make_identity(nc, identity)
        identity_bf = const.tile([P, P], bf16)
        nc.vector.tensor_copy(identity_bf[:], identity[:])
        ones_bf = const.tile([P, HL, 1], bf16)
        nc.vector.memset(ones_bf[:], 1.0)
        # head-select masks: hmask[0:2, s, :] is 1 on partition s, else 0.
        # K=2 lhsT for broadcasting one pair-row's denominator to 64
        # partitions.
        hmask_f32 = const.tile([2, 2, 64], f32)
        nc.gpsimd.memset(hmask_f32[:], 0.0)
        nc.gpsimd.affine_select(
            out=hmask_f32[:],
            in_=hmask_f32[:],
            compare_op=mybir.AluOpType.not_equal,
            fill=1.0,
            base=0,
            pattern=[[-1, 2], [0, 64]],
            channel_multiplier=1,
        )


        # ---- persistent activations ----
        act_pool = ctx.enter_context(tc.tile_pool(name="acts", bufs=1))
        qt = [act_pool.tile([P, N], bf16, name=f"qt{p}", tag=f"qt{p}")
              for p in range(NPAIR)]
        kt = [act_pool.tile([P, M], bf16, name=f"kt{p}", tag=f"kt{p}")
              for p in range(NPAIR)]
        v_all = [act_pool.tile([P, HL, 65], bf16, name=f"v{t}", tag=f"v{t}")
                 for t in range(MT)]
        ctxn = [act_pool.tile([P, N], bf16, name=f"ctxn{p}", tag=f"ctxn{p}")
                for p in range(NPAIR)]
        wo_sb = act_pool.tile([P, NPAIR, OUT], bf16, name="wo_sb", tag="wo_sb")
        nc.sync.dma_start(wo_sb[:], wo[:, :, :])

        # ---- phase 1: load + V projection + pair-0 Q/K projections ----
        # Q/K projections for pairs 1-3 are deferred into the attention
        # phase as micro-ops (one matmul per attention m-tile iteration)
        # so the softmax engine starts ~35us earlier.
        x_pool = ctx.enter_context(tc.tile_pool(name="x_sb", bufs=2))
        w_pool = ctx.enter_context(tc.tile_pool(name="w_sb", bufs=2))

        def load_xw(x_dram, w_dram):
            # per-d-slice x tiles so matmuls start as soon as each
            # slice's DMA lands
            x_sb = [x_pool.tile([P, N], bf16, name=f"x{d}", tag=f"x{d}")
                    for d in range(DT)]
            w_sb = w_pool.tile([P, DT, HL * HS], bf16, name="w_sb", tag="w_sb")
            for dt_i in range(DT):
                dsl = slice(dt_i * P, (dt_i + 1) * P)
                nc.sync.dma_start(x_sb[dt_i][:], x_dram[dsl, :])
                nc.sync.dma_start(w_sb[:, dt_i, :], w_dram[dsl, :])
            return x_sb, w_sb

        # V first (attention needs all of v_all).
        xv_sb, wv_sb = load_xw(xv, wv)
        with tc.tile_pool(name="pv_psum", bufs=3, space="PSUM") as pv_psum:
            for t in range(MT):
                ps = pv_psum.tile([P, HL * HS], f32, name="v_ps", tag="v_ps")
                for dt_i in range(DT):
                    nc.tensor.matmul(
                        ps[:],
                        xv_sb[dt_i][:, t * P:(t + 1) * P],
                        wv_sb[:, dt_i, :],
                        start=(dt_i == 0), stop=(dt_i == DT - 1),
                    )
                nc.vector.tensor_copy(
                    v_all[t][:, :, 0:64], ps[:].rearrange("p (h o) -> p h o", h=HL))
                nc.vector.tensor_copy(v_all[t][:, :, 64:65], ones_bf[:])

        xq_sb, wq_sb = load_xw(xq, wq)
        xk_sb, wk_sb = load_xw(xk, wk)

        with tc.tile_pool(name="pj_psum", bufs=2, space="PSUM") as pj_psum:
            for x_sb, w_sb, dst in ((xq_sb, wq_sb, qt), (xk_sb, wk_sb, kt)):
                for half in range(2):
                    hsl = slice(half * 1024, (half + 1) * 1024)
                    ps = pj_psum.tile([P, 1024], f32, name="qk_ps", tag="qk_ps")
                    for dt_i in range(DT):
                        for cc in range(2):
                            nc.tensor.matmul(
                                ps[:, cc * C:(cc + 1) * C],
                                w_sb[:, dt_i, 0:P],
                                x_sb[dt_i][:, half * 1024 + cc * C:
                                           half * 1024 + (cc + 1) * C],
                                start=(dt_i == 0), stop=(dt_i == DT - 1),
                            )
                    nc.vector.tensor_copy(dst[0][:, hsl], ps[:])

        # ---- phase 2: attention (software-pipelined across chunks) ----
        # Deferred Q/K projection micro-ops: pairs 1-3, one [P, 512]
        # quarter per 9 thunks (8 accumulating matmuls + 1 eviction),
        # drip-fed one thunk per attention m-tile so the Exp engine
        # never starves while the remaining projections complete.
        micro = []

        def make_quarter(x_sb, w_sb, dst, pp, cc):
            state = {}

            def mm(dt_i):
                def run():
                    if dt_i == 0:
                        state["ps"] = work_psum.tile(
                            [P, C], f32, name="aux", tag="aux", bufs=1)
                    nc.tensor.matmul(
                        state["ps"][:],
                        w_sb[:, dt_i, pp * P:(pp + 1) * P],
                        x_sb[dt_i][:, cc * C:(cc + 1) * C],
                        start=(dt_i == 0), stop=(dt_i == DT - 1),
                    )
                return run

            def evict():
                nc.vector.tensor_copy(
                    dst[pp][:, cc * C:(cc + 1) * C], state.pop("ps")[:])
            return [mm(d) for d in range(DT)] + [evict]

        for pp in range(1, NPAIR):
            for x_sb, w_sb, dst in ((xk_sb, wk_sb, kt), (xq_sb, wq_sb, qt)):
                for cc in range(NC):
                    micro.extend(make_quarter(x_sb, w_sb, dst, pp, cc))

        PIPE = 3
        with tc.tile_pool(name="et", bufs=6) as et_pool, \
             tc.tile_pool(name="tmp", bufs=3) as tmp_pool, \
             tc.tile_pool(name="lg_psum", bufs=2, space="PSUM") as lg_psum, \
             tc.tile_pool(name="work_psum", bufs=1, space="PSUM") as work_psum:

            def make_evict(p, c, cps, sums):
                # Evict closures: ctx rows UN-normalized into pair-stacked
                # ctxn (odd head via bf16 PE shift to partitions 64:128);
                # denominator rows staged to sums via partition-hop DMA.
                # Emitted DURING the next chunk's t-loop so the PE / ACT
                # pipeline never drains at a chunk boundary. Split in two:
                # the DVE part frees the cps banks ASAP, the PE shift part
                # runs later.
                csl = slice(c * C, (c + 1) * C)
                tmp = tmp_pool.tile([64, C], bf16, name="ctmp", tag="ctmp")

                def evict_dve():
                    stage = tmp_pool.tile([P, 2, C], f32, name="sstage", tag="sstage")
                    nc.vector.tensor_copy(stage[64:65, 0, :], cps[0][64:65, :])
                    nc.vector.tensor_copy(stage[64:65, 1, :], cps[1][64:65, :])
                    nc.sync.dma_start(sums[0:1, csl], stage[64:65, 0, :])
                    nc.sync.dma_start(sums[1:2, csl], stage[64:65, 1, :])
                    nc.vector.tensor_copy(ctxn[p][0:64, csl], cps[0][0:64, :])
                    nc.vector.tensor_copy(tmp[:], cps[1][0:64, :])

                def evict_shift():
                    sh = work_psum.tile([P, C], f32, name="shbc", tag="shbc",
                                        bufs=1)
                    nc.tensor.matmul(
                        sh[64:128, :], identity_bf[0:64, 0:64], tmp[:],
                        start=True, stop=True)
                    nc.vector.tensor_copy(ctxn[p][64:128, csl], sh[64:128, :])
                return [evict_dve, evict_shift]

            def make_norm(p, sums):
                # Per-pair deferred softmax normalization, split into
                # 512-wide quarters: reciprocal of the pair's denominator
                # rows, PE broadcast to 64 partitions via pair-row select
                # masks, multiply into ctxn. Emitted during the NEXT
                # pair's attention (PE slack absorbs it).
                def quarter(cc, recip):
                    qsl = slice(cc * C, (cc + 1) * C)

                    def norm_q():
                        if recip:
                            nc.vector.reciprocal(sums[:], sums[:])
                        bc = work_psum.tile([P, C], f32, name="shbc",
                                            tag="shbc", bufs=1)
                        for s in range(2):
                            nc.tensor.matmul(
                                bc[s * 64:(s + 1) * 64, :],
                                hmask_f32[:, s, :],
                                sums[:, qsl],
                                start=True, stop=True,
                            )
                        nc.vector.tensor_mul(
                            ctxn[p][:, qsl], ctxn[p][:, qsl], bc[:])
                    return norm_q
                return [quarter(cc, cc == 0) for cc in range(NC)]

            # two deferred-work queues: crit frees ctx PSUM banks (must
            # run early each chunk); bulk absorbs shifts + normalization
            # in PE slack.
            pending_crit = []
            pending_bulk = []
            BULK_SLOTS = (5, 10, 14)

            sums_of = {}
            for p in range(NPAIR):
                hA, hB = 2 * p, 2 * p + 1
                sums_of[p] = tmp_pool.tile([2, N], f32, name="sums",
                                           tag="sums", bufs=2)
                for c in range(NC):
                    csl = slice(c * C, (c + 1) * C)
                    cps = {
                        0: work_psum.tile([65, C], f32, name="cpsA", tag="cps",
                                          bufs=2),
                        1: work_psum.tile([65, C], f32, name="cpsB", tag="cps",
                                          bufs=2),
                    }
                    ets = {}

                    def emit_logits(t):
                        tsl = slice(t * P, (t + 1) * P)
                        lg = lg_psum.tile([P, 2, C], f32, name="lg", tag="lg")
                        nc.tensor.matmul(
                            lg[:, 0, :], kt[p][0:64, tsl], qt[p][0:64, csl],
                            start=True, stop=True)
                        nc.tensor.matmul(
                            lg[:, 1, :], kt[p][64:128, tsl], qt[p][64:128, csl],
                            start=True, stop=True)
                        et = et_pool.tile([P, 2, C], bf16, name="et", tag="et")
                        nc.scalar.activation(
                            et[:], lg[:], mybir.ActivationFunctionType.Exp,
                            scale=0.125)
                        ets[t] = et

                    def emit_ctx(t):
                        et = ets.pop(t)
                        nc.tensor.matmul(
                            cps[0][:], v_all[t][:, hA, :], et[:, 0, :],
                            start=(t == 0), stop=(t == MT - 1))
                        nc.tensor.matmul(
                            cps[1][:], v_all[t][:, hB, :], et[:, 1, :],
                            start=(t == 0), stop=(t == MT - 1))

                    for t in range(MT):
                        emit_logits(t)
                        if t == 1 and pending_crit:
                            pending_crit.pop(0)()
                        elif t in BULK_SLOTS and pending_bulk:
                            pending_bulk.pop(0)()
                        if micro:
                            micro.pop(0)()
                        if t >= PIPE:
                            emit_ctx(t - PIPE)
                    for t in range(MT - PIPE, MT):
                        emit_ctx(t)

                    ev = make_evict(p, c, cps, sums_of[p])
                    pending_crit.append(ev[0])
                    pending_bulk.append(ev[1])
                if p > 0:
                    pending_bulk.extend(make_norm(p - 1, sums_of[p - 1]))
            for w in micro:
                w()
            for w in pending_crit:
                w()
            for w in pending_bulk:
                w()
            for w in make_norm(NPAIR - 1, sums_of[NPAIR - 1]):
                w()

        # ---- phase 3: output projection (transposed: outT = Wo^T ctx) ----
        # stationary = wo column block [128=(s,hs), 128], reused across the
        # four n-chunks (4 matmuls per LDWEIGHTS); accumulate over pairs.
        # Host transposes [OUT, N] -> [N, OUT].
        with tc.tile_pool(name="out_psum", bufs=8, space="PSUM") as out_psum, \
             tc.tile_pool(name="out_sb", bufs=2) as out_pool:
            for oc in range(OUT // P):
                ops = [out_psum.tile([P, C], f32, name=f"ops{nk}", tag="ops")
                       for nk in range(NC)]
                for p in range(NPAIR):
                    for nk in range(NC):
                        nc.tensor.matmul(
                            ops[nk][:],
                            wo_sb[:, p, oc * P:(oc + 1) * P],
                            ctxn[p][:, nk * C:(nk + 1) * C],
                            start=(p == 0), stop=(p == NPAIR - 1),
                        )
                ot = out_pool.tile([P, N], f32, name="ot", tag="ot")
                for nk in range(NC):
                    nc.vector.tensor_copy(ot[:, nk * C:(nk + 1) * C], ops[nk][:])
                nc.sync.dma_start(out_ap[oc * P:(oc + 1) * P, :], ot[:])


def build_nc():
    import concourse.bacc as bacc
    import concourse.tile as tile
    from concourse import mybir

    nc = bacc.Bacc("TRN2", target_bir_lowering=False, debug=False)
    f32 = mybir.dt.float32
    bf16 = mybir.dt.bfloat16
    ins = {
        "xq": nc.dram_tensor("xq", (D, N), bf16, kind="ExternalInput").ap(),
        "xk": nc.dram_tensor("xk", (D, M), bf16, kind="ExternalInput").ap(),
        "xv": nc.dram_tensor("xv", (D, M), bf16, kind="ExternalInput").ap(),
        "wq": nc.dram_tensor("wq", (D, HL * HS), bf16, kind="ExternalInput").ap(),
        "wk": nc.dram_tensor("wk", (D, HL * HS), bf16, kind="ExternalInput").ap(),
        "wv": nc.dram_tensor("wv", (D, HL * HS), bf16, kind="ExternalInput").ap(),
        "wo": nc.dram_tensor("wo", (P, NPAIR, OUT), bf16, kind="ExternalInput").ap(),
    }
    out_ap = nc.dram_tensor("out", (OUT, N), f32, kind="ExternalOutput").ap()
    with tile.TileContext(nc) as tc:
        build_mha(tc, ins, out_ap)
    nc.compile()
    return nc


def make_in_maps(inputs):
    import ml_dtypes
    bf16 = ml_dtypes.bfloat16

    q = np.asarray(inputs["query"], dtype=np.float32)
    k = np.asarray(inputs["key"], dtype=np.float32)
    v = np.asarray(inputs["value"], dtype=np.float32)
    wq = np.asarray(inputs["query_kernel"], dtype=np.float32)
    wk = np.asarray(inputs["key_kernel"], dtype=np.float32)
    wv = np.asarray(inputs["value_kernel"], dtype=np.float32)
    wo = np.asarray(inputs["projection_kernel"], dtype=np.float32)

    # [H, D, HS] -> per head-group [D, HL*HS] bf16
    def wlay(w, hs):
        return np.ascontiguousarray(
            w[hs].transpose(1, 0, 2).reshape(D, HL * HS)).astype(bf16)

    # [H, HS, OUT] -> per head-group [128=(s,o), NPAIR, OUT] bf16
    def wolay(w, hs):
        return np.ascontiguousarray(
            w[hs].reshape(NPAIR, 2, HS, OUT).transpose(1, 2, 0, 3)
            .reshape(P, NPAIR, OUT)).astype(bf16)

    in_maps = []
    for cc in range(8):
        b, hg = divmod(cc, 2)
        hs = slice(hg * HL, (hg + 1) * HL)
        in_maps.append({
            "xq": np.ascontiguousarray(q[b].T).astype(bf16),
            "xk": np.ascontiguousarray(k[b].T).astype(bf16),
            "xv": np.ascontiguousarray(v[b].T).astype(bf16),
            "wq": wlay(wq, hs),
            "wk": wlay(wk, hs),
            "wv": wlay(wv, hs),
            "wo": wolay(wo, hs),
        })
    return in_maps


def combine(results, bias):
    # per-core output is transposed [OUT, N]
    out = np.empty((B, N, OUT), dtype=np.float32)
    for b in range(B):
        out[b] = (results[2 * b]["out"] + results[2 * b + 1]["out"]).T
    out += np.asarray(bias, dtype=np.float32)[None, None, :]
    return out


_NC_CACHE = None
_LDW_PATCHED = False


def _enable_ldw_opt():
    """No-op: walrus --enable-ldw-opt=true rejects tile_position'd
    LDWEIGHTS ("InstLdweights is not compatible with LDW optimization"),
    and this kernel's row-tiled attention matmuls need tile positions.
    Kept for test.py compatibility."""
    return


def kernel(**inputs):
    global _NC_CACHE
    from concourse import bass_utils
    _enable_ldw_opt()

    if _NC_CACHE is None:
        _NC_CACHE = build_nc()
    nc = _NC_CACHE
    in_maps = make_in_maps(inputs)
    res = bass_utils.run_bass_kernel_spmd(nc, in_maps, core_ids=list(range(8)))
    return combine(res.results, inputs["projection_bias"])


# revision 21
# speedup vs baseline: 1.1841x; 1.0831x over previous
"""Multi-head attention forward on 8 Trainium2 NeuronCores.

Problem (hardcoded): B=4, N=M=2048, D=1024, H=16, HS=64, OUT=1024, fp32.

Sharding: 8 cores = 4 batches x 2 head-groups of 8 heads. Each core
computes a partial output [2048, 1024] = sum over its 8 heads of
softmax((X_q Wq_h)(X_k Wk_h)^T / 8) (X_v Wv_h) Wo_h.  Host sums the two
head-group partials per batch and adds the projection bias.

Host-side prep: x tensors are transposed to [D, N] and converted to
bf16 (so no on-chip transposes are needed); W's are pre-arranged into
the SBUF layouts and converted to bf16.

Per-core kernel:
  1. QKV projections: stationary = W pair-column [d-slice, 128] reused
     across four F=512 moving chunks of xT (amortizes LDWEIGHTS);
     V projection: stationary = xT m-tile, moving = Wv [d-slice, 512].
     PSUM f32 accumulate over 8 d-slices, evict to bf16:
     qt/kt pair-stacked [128, 2048], v_all [128, 8 heads, 65] with a
     ones column at 64 (softmax denominator).
  2. Attention per (pair, 512-chunk): the two heads of a pair run
     CONCURRENTLY as row-tiled matmuls (tile positions (0,0)/(64,0):
     head A uses PE rows 0-63, head B rows 64-127) into adjacent PSUM
     banks; ONE Exp activation [128, 2, 512] (F=1024) per m-tile covers
     both heads; ctx accumulation per head with the [v|1] stationary.
     Pace is set by ScalarE's Exp (~1.15us per m-tile).
  3. Deferred softmax normalization: one reciprocal over all head
     denominators, PE broadcast via head-select masks, DVE multiply.
  4. Output projection: lhsT = ctxn pair n-block (K=128), rhs = Wo pair,
     stationary reused across both 512-wide output chunks.
"""

import os
import sys

import numpy as np

for _p in ("/opt/trn_rl_repo",):
    if _p not in sys.path and os.path.isdir(_p):
        sys.path.insert(0, _p)

B, N, M, D = 4, 2048, 2048, 1024
H, HS, OUT = 16, 64, 1024
HL = 8           # heads per core
P = 128
NPAIR = HL // 2  # head pairs per core
DT = D // P      # 8 d-tiles
NT = N // P      # 16 n-tiles
MT = M // P      # 16 m-tiles
C = 512          # attention n-chunk width
NC = N // C      # 4 chunks


def build_mha(tc, ins, out_ap):
    import contextlib

    from concourse import mybir

    nc = tc.nc
    f32 = mybir.dt.float32
    f32r = mybir.dt.float32r
    bf16 = mybir.dt.bfloat16

    xq, xk, xv = ins["xq"], ins["xk"], ins["xv"]
    wq, wk, wv, wo = ins["wq"], ins["wk"], ins["wv"], ins["wo"]

    with contextlib.ExitStack() as ctx:
        # ---- constant tiles ----
        const = ctx.enter_context(tc.tile_pool(name="const", bufs=1))
        identity = const.tile([P, P], f32)
        from concourse.masks import make_identity
        make_identity(nc, identity)
        identity_bf = const.tile([P, P], bf16)
        nc.vector.tensor_copy(identity_bf[:], identity[:])
        ones_bf = const.tile([P, HL, 1], bf16)
        nc.vector.memset(ones_bf[:], 1.0)
        # head-select masks: hmask[0:2, s, :] is 1 on partition s, else 0.
        # K=2 lhsT for broadcasting one pair-row's denominator to 64
        # partitions.
        hmask_f32 = const.tile([2, 2, 64], f32)
        nc.gpsimd.memset(hmask_f32[:], 0.0)
        nc.gpsimd.affine_select(
            out=hmask_f32[:],
            in_=hmask_f32[:],
            compare_op=mybir.AluOpType.not_equal,
            fill=1.0,
            base=0,
            pattern=[[-1, 2], [0, 64]],
            channel_multiplier=1,
        )


        # ---- persistent activations ----
        act_pool = ctx.enter_context(tc.tile_pool(name="acts", bufs=1))
        qt = [act_pool.tile([P, N], bf16, name=f"qt{p}", tag=f"qt{p}")
              for p in range(NPAIR)]
        kt = [act_pool.tile([P, M], bf16, name=f"kt{p}", tag=f"kt{p}")
              for p in range(NPAIR)]
        v_all = [act_pool.tile([P, HL, 65], bf16, name=f"v{t}", tag=f"v{t}")
                 for t in range(MT)]
        ctxn = [act_pool.tile([P, N], bf16, name=f"ctxn{p}", tag=f"ctxn{p}")
                for p in range(NPAIR)]
        wo_sb = act_pool.tile([P, NPAIR, OUT], bf16, name="wo_sb", tag="wo_sb")
        nc.sync.dma_start(wo_sb[:], wo[:, :, :])

        # ---- phase 1: load + V projection + pair-0 Q/K projections ----
        # Q/K projections for pairs 1-3 are deferred into the attention
        # phase as micro-ops (one matmul per attention m-tile iteration)
        # so the softmax engine starts ~35us earlier.
        x_pool = ctx.enter_context(tc.tile_pool(name="x_sb", bufs=2))
        w_pool = ctx.enter_context(tc.tile_pool(name="w_sb", bufs=2))

        def load_xw(x_dram, w_dram):
            # per-d-slice x tiles so matmuls start as soon as each
            # slice's DMA lands
            x_sb = [x_pool.tile([P, N], bf16, name=f"x{d}", tag=f"x{d}")
                    for d in range(DT)]
            w_sb = w_pool.tile([P, DT, HL * HS], bf16, name="w_sb", tag="w_sb")
            for dt_i in range(DT):
                dsl = slice(dt_i * P, (dt_i + 1) * P)
                nc.sync.dma_start(x_sb[dt_i][:], x_dram[dsl, :])
                nc.sync.dma_start(w_sb[:, dt_i, :], w_dram[dsl, :])
            return x_sb, w_sb

        # V first (attention needs all of v_all).
        xv_sb, wv_sb = load_xw(xv, wv)
        with tc.tile_pool(name="pv_psum", bufs=3, space="PSUM") as pv_psum:
            for t in range(MT):
                ps = pv_psum.tile([P, HL * HS], f32, name="v_ps", tag="v_ps")
                for dt_i in range(DT):
                    nc.tensor.matmul(
                        ps[:],
                        xv_sb[dt_i][:, t * P:(t + 1) * P],
                        wv_sb[:, dt_i, :],
                        start=(dt_i == 0), stop=(dt_i == DT - 1),
                    )
                nc.vector.tensor_copy(
                    v_all[t][:, :, 0:64], ps[:].rearrange("p (h o) -> p h o", h=HL))
                nc.vector.tensor_copy(v_all[t][:, :, 64:65], ones_bf[:])

        xq_sb, wq_sb = load_xw(xq, wq)
        xk_sb, wk_sb = load_xw(xk, wk)

        with tc.tile_pool(name="pj_psum", bufs=2, space="PSUM") as pj_psum:
            for x_sb, w_sb, dst in ((xq_sb, wq_sb, qt), (xk_sb, wk_sb, kt)):
                for half in range(2):
                    hsl = slice(half * 1024, (half + 1) * 1024)
                    ps = pj_psum.tile([P, 1024], f32, name="qk_ps", tag="qk_ps")
                    for dt_i in range(DT):
                        for cc in range(2):
                            nc.tensor.matmul(
                                ps[:, cc * C:(cc + 1) * C],
                                w_sb[:, dt_i, 0:P],
                                x_sb[dt_i][:, half * 1024 + cc * C:
                                           half * 1024 + (cc + 1) * C],
                                start=(dt_i == 0), stop=(dt_i == DT - 1),
                            )
                    nc.vector.tensor_copy(dst[0][:, hsl], ps[:])

        # ---- phase 2: attention (software-pipelined across chunks) ----
        # Deferred Q/K projection micro-ops: pairs 1-3, one [P, 512]
        # quarter per 9 thunks (8 accumulating matmuls + 1 eviction),
        # drip-fed one thunk per attention m-tile so the Exp engine
        # never starves while the remaining projections complete.
        micro = []

        def make_quarter(x_sb, w_sb, dst, pp, cc):
            state = {}

            def mm(dt_i):
                def run():
                    if dt_i == 0:
                        state["ps"] = work_psum.tile(
                            [P, C], f32, name="aux", tag="aux", bufs=1)
                    nc.tensor.matmul(
                        state["ps"][:],
                        w_sb[:, dt_i, pp * P:(pp + 1) * P],
                        x_sb[dt_i][:, cc * C:(cc + 1) * C],
                        start=(dt_i == 0), stop=(dt_i == DT - 1),
                    )
                return run

            def evict():
                nc.vector.tensor_copy(
                    dst[pp][:, cc * C:(cc + 1) * C], state.pop("ps")[:])
            return [mm(d) for d in range(DT)] + [evict]

        for pp in range(1, NPAIR):
            for x_sb, w_sb, dst in ((xk_sb, wk_sb, kt), (xq_sb, wq_sb, qt)):
                for cc in range(NC):
                    micro.extend(make_quarter(x_sb, w_sb, dst, pp, cc))

        PIPE = 3
        with tc.tile_pool(name="et", bufs=6) as et_pool, \
             tc.tile_pool(name="tmp", bufs=3) as tmp_pool, \
             tc.tile_pool(name="lg_psum", bufs=2, space="PSUM") as lg_psum, \
             tc.tile_pool(name="work_psum", bufs=1, space="PSUM") as work_psum:

            def make_evict(p, c, cps, sums):
                # Evict closures: ctx rows UN-normalized into pair-stacked
                # ctxn (odd head via bf16 PE shift to partitions 64:128);
                # denominator rows staged to sums via partition-hop DMA.
                # Emitted DURING the next chunk's t-loop so the PE / ACT
                # pipeline never drains at a chunk boundary. Split in two:
                # the DVE part frees the cps banks ASAP, the PE shift part
                # runs later.
                csl = slice(c * C, (c + 1) * C)
                tmp = tmp_pool.tile([64, C], bf16, name="ctmp", tag="ctmp")

                def evict_dve():
                    stage = tmp_pool.tile([P, 2, C], f32, name="sstage", tag="sstage")
                    nc.vector.tensor_copy(stage[64:65, 0, :], cps[0][64:65, :])
                    nc.vector.tensor_copy(stage[64:65, 1, :], cps[1][64:65, :])
                    nc.sync.dma_start(sums[0:1, csl], stage[64:65, 0, :])
                    nc.sync.dma_start(sums[1:2, csl], stage[64:65, 1, :])
                    nc.vector.tensor_copy(ctxn[p][0:64, csl], cps[0][0:64, :])
                    nc.vector.tensor_copy(tmp[:], cps[1][0:64, :])

                def evict_shift():
                    sh = work_psum.tile([P, C], f32, name="shbc", tag="shbc",
                                        bufs=1)
                    nc.tensor.matmul(
                        sh[64:128, :], identity_bf[0:64, 0:64], tmp[:],
                        start=True, stop=True)
                    nc.vector.tensor_copy(ctxn[p][64:128, csl], sh[64:128, :])
                return [evict_dve, evict_shift]

            def make_norm(p, sums):
                # Per-pair deferred softmax normalization, split into
                # 512-wide quarters: reciprocal of the pair's denominator
                # rows, PE broadcast to 64 partitions via pair-row select
                # masks, multiply into ctxn. Emitted during the NEXT
                # pair's attention (PE slack absorbs it).
                def quarter(cc, recip):
                    qsl = slice(cc * C, (cc + 1) * C)

                    def norm_q():
                        if recip:
                            nc.vector.reciprocal(sums[:, qsl], sums[:, qsl])
                        bc = work_psum.tile([P, C], f32, name="shbc",
                                            tag="shbc", bufs=1)
                        for s in range(2):
                            nc.tensor.matmul(
                                bc[s * 64:(s + 1) * 64, :],
                                hmask_f32[:, s, :],
                                sums[:, qsl],
                                start=True, stop=True,
                            )
                        nc.vector.tensor_mul(
                            ctxn[p][:, qsl], ctxn[p][:, qsl], bc[:])
                    return norm_q
                return [quarter(cc, True) for cc in range(NC)]

            # two deferred-work queues: crit frees ctx PSUM banks (must
            # run early each chunk); bulk absorbs shifts + normalization
            # in PE slack.
            pending_crit = []
            pending_bulk = []
            BULK_SLOTS = (5, 10, 14)

            sums_of = {}
            norm_quarters = {}
            for p in range(NPAIR):
                hA, hB = 2 * p, 2 * p + 1
                sums_of[p] = tmp_pool.tile([2, N], f32, name="sums",
                                           tag="sums", bufs=2)
                norm_quarters[p] = make_norm(p, sums_of[p])
                for c in range(NC):
                    csl = slice(c * C, (c + 1) * C)
                    cps = {
                        0: work_psum.tile([65, C], f32, name="cpsA", tag="cps",
                                          bufs=2),
                        1: work_psum.tile([65, C], f32, name="cpsB", tag="cps",
                                          bufs=2),
                    }
                    ets = {}

                    def emit_logits(t):
                        tsl = slice(t * P, (t + 1) * P)
                        lg = lg_psum.tile([P, 2, C], f32, name="lg", tag="lg")
                        nc.tensor.matmul(
                            lg[:, 0, :], kt[p][0:64, tsl], qt[p][0:64, csl],
                            start=True, stop=True)
                        nc.tensor.matmul(
                            lg[:, 1, :], kt[p][64:128, tsl], qt[p][64:128, csl],
                            start=True, stop=True)
                        et = et_pool.tile([P, 2, C], bf16, name="et", tag="et")
                        nc.scalar.activation(
                            et[:], lg[:], mybir.ActivationFunctionType.Exp,
                            scale=0.125)
                        ets[t] = et

                    def emit_ctx(t):
                        et = ets.pop(t)
                        nc.tensor.matmul(
                            cps[0][:], v_all[t][:, hA, :], et[:, 0, :],
                            start=(t == 0), stop=(t == MT - 1))
                        nc.tensor.matmul(
                            cps[1][:], v_all[t][:, hB, :], et[:, 1, :],
                            start=(t == 0), stop=(t == MT - 1))

                    for t in range(MT):
                        emit_logits(t)
                        if t == 1 and pending_crit:
                            pending_crit.pop(0)()
                        elif t in BULK_SLOTS and pending_bulk:
                            pending_bulk.pop(0)()
                        if micro and t >= 2:
                            micro.pop(0)()
                        if micro and t in (8, 12):
                            micro.pop(0)()
                        if t >= PIPE:
                            emit_ctx(t - PIPE)
                    for t in range(MT - PIPE, MT):
                        emit_ctx(t)

                    ev = make_evict(p, c, cps, sums_of[p])
                    pending_crit.append(ev[0])
                    pending_bulk.append(ev[1])
                if p > 0:
                    pending_bulk.extend(norm_quarters[p - 1])
            for w in micro:
                w()
            for w in pending_crit:
                w()
            for w in pending_bulk:
                w()
            for w in norm_quarters[NPAIR - 1]:
                w()

        # ---- phase 3: output projection (transposed: outT = Wo^T ctx) ----
        # stationary = wo column block [128=(s,hs), 128], reused across the
        # four n-chunks (4 matmuls per LDWEIGHTS); accumulate over pairs.
        # Host transposes [OUT, N] -> [N, OUT].
        with tc.tile_pool(name="out_psum", bufs=8, space="PSUM") as out_psum, \
             tc.tile_pool(name="out_sb", bufs=2) as out_pool:
            for oc in range(OUT // P):
                ops = [out_psum.tile([P, C], f32, name=f"ops{nk}", tag="ops")
                       for nk in range(NC)]
                for p in range(NPAIR):
                    for nk in range(NC):
                        nc.tensor.matmul(
                            ops[nk][:],
                            wo_sb[:, p, oc * P:(oc + 1) * P],
                            ctxn[p][:, nk * C:(nk + 1) * C],
                            start=(p == 0), stop=(p == NPAIR - 1),
                        )
                ot = out_pool.tile([P, N], f32, name="ot", tag="ot")
                for nk in range(NC):
                    nc.vector.tensor_copy(ot[:, nk * C:(nk + 1) * C], ops[nk][:])
                nc.sync.dma_start(out_ap[oc * P:(oc + 1) * P, :], ot[:])


def build_nc():
    import concourse.bacc as bacc
    import concourse.tile as tile
    from concourse import mybir

    nc = bacc.Bacc("TRN2", target_bir_lowering=False, debug=False)
    f32 = mybir.dt.float32
    bf16 = mybir.dt.bfloat16
    ins = {
        "xq": nc.dram_tensor("xq", (D, N), bf16, kind="ExternalInput").ap(),
        "xk": nc.dram_tensor("xk", (D, M), bf16, kind="ExternalInput").ap(),
        "xv": nc.dram_tensor("xv", (D, M), bf16, kind="ExternalInput").ap(),
        "wq": nc.dram_tensor("wq", (D, HL * HS), bf16, kind="ExternalInput").ap(),
        "wk": nc.dram_tensor("wk", (D, HL * HS), bf16, kind="ExternalInput").ap(),
        "wv": nc.dram_tensor("wv", (D, HL * HS), bf16, kind="ExternalInput").ap(),
        "wo": nc.dram_tensor("wo", (P, NPAIR, OUT), bf16, kind="ExternalInput").ap(),
    }
    out_ap = nc.dram_tensor("out", (OUT, N), f32, kind="ExternalOutput").ap()
    with tile.TileContext(nc) as tc:
        build_mha(tc, ins, out_ap)
    nc.compile()
    return nc


def make_in_maps(inputs):
    import ml_dtypes
    bf16 = ml_dtypes.bfloat16

    q = np.asarray(inputs["query"], dtype=np.float32)
    k = np.asarray(inputs["key"], dtype=np.float32)
    v = np.asarray(inputs["value"], dtype=np.float32)
    wq = np.asarray(inputs["query_kernel"], dtype=np.float32)
    wk = np.asarray(inputs["key_kernel"], dtype=np.float32)
    wv = np.asarray(inputs["value_kernel"], dtype=np.float32)
    wo = np.asarray(inputs["projection_kernel"], dtype=np.float32)

    # [H, D, HS] -> per head-group [D, HL*HS] bf16
    def wlay(w, hs):
        return np.ascontiguousarray(
            w[hs].transpose(1, 0, 2).reshape(D, HL * HS)).astype(bf16)

    # [H, HS, OUT] -> per head-group [128=(s,o), NPAIR, OUT] bf16
    def wolay(w, hs):
        return np.ascontiguousarray(
            w[hs].reshape(NPAIR, 2, HS, OUT).transpose(1, 2, 0, 3)
            .reshape(P, NPAIR, OUT)).astype(bf16)

    in_maps = []
    for cc in range(8):
        b, hg = divmod(cc, 2)
        hs = slice(hg * HL, (hg + 1) * HL)
        in_maps.append({
            "xq": np.ascontiguousarray(q[b].T).astype(bf16),
            "xk": np.ascontiguousarray(k[b].T).astype(bf16),
            "xv": np.ascontiguousarray(v[b].T).astype(bf16),
            "wq": wlay(wq, hs),
            "wk": wlay(wk, hs),
            "wv": wlay(wv, hs),
            "wo": wolay(wo, hs),
        })
    return in_maps


def combine(results, bias):
    # per-core output is transposed [OUT, N]
    out = np.empty((B, N, OUT), dtype=np.float32)
    for b in range(B):
        out[b] = (results[2 * b]["out"] + results[2 * b + 1]["out"]).T
    out += np.asarray(bias, dtype=np.float32)[None, None, :]
    return out


_NC_CACHE = None
_LDW_PATCHED = False


def _enable_ldw_opt():
    """No-op: walrus --enable-ldw-opt=true rejects tile_position'd
    LDWEIGHTS ("InstLdweights is not compatible with LDW optimization"),
    and this kernel's row-tiled attention matmuls need tile positions.
    Kept for test.py compatibility."""
    return


def kernel(**inputs):
    global _NC_CACHE
    from concourse import bass_utils
    _enable_ldw_opt()

    if _NC_CACHE is None:
        _NC_CACHE = build_nc()
    nc = _NC_CACHE
    in_maps = make_in_maps(inputs)
    res = bass_utils.run_bass_kernel_spmd(nc, in_maps, core_ids=list(range(8)))
    return combine(res.results, inputs["projection_bias"])
